# revision 19
# baseline (speedup 1.0000x reference)
"""Trainium2 Bass kernel for nn_AttnBlock (GroupNorm + single-head-split attention + residual).

Sharding: 8 cores = (batch b in {0,1}) x (head h in {0..3}).  Each core computes
the full attention for its (b, h) pair plus the partial output projection
wo[:, head_cols] @ att_out_head -> [512, 4096].  Host sums the 4 head partials
per batch, adds the residual x and output bias bo.

Per-core kernel (all fp32):
  1. GroupNorm(32 groups) of x[b] [512, 4096]  (channels on partitions, 4 chunks of 128)
  2. q = wq_h @ h_, k = wk_h @ h_, v = wv_h @ h_   ([128, 4096] each; head slices only)
  3. vT via PE transposes ([4096, 128] as 32 tiles)
  4. For each 512-query group: S^T[j,i] = k^T q (32 j-tiles), P = exp(scale*S^T) (ACT),
     denominators via ones-matmul, out^T = V P (PSUM accumulation), normalize with
     gpsimd partition_broadcast of 1/den, final partial = wo_h^T @ out^T.
"""

import sys

sys.path.insert(0, "/opt/trn_rl_repo")

import numpy as np

C = 512
HEADS = 4
HC = 128          # head channels
N = 4096          # h*w pixels
P = 128           # partitions
NCH = C // P      # 4 channel chunks
NJT = N // P      # 32 key tiles
IG = 512          # query-group width
NIG = N // IG     # 8 query groups
GSIZE = 16        # channels per groupnorm group
NGRP = 32
EPS = 1e-6
SCALE = float(C) ** -0.5
DEN_LAG = 3       # delay (in j-tiles) of the denominator matmul behind the S^T matmul

_NC_CACHE = {}


def _build_nc():
    from contextlib import ExitStack

    import concourse.bacc as bacc
    import concourse.bass as bass
    import concourse.tile as tile
    from concourse import mybir
    from concourse.masks import make_identity

    f32 = mybir.dt.float32
    f32r = mybir.dt.float32r

    def r(ap):
        return ap.bitcast(f32r)
    AF = mybir.ActivationFunctionType
    OP = mybir.AluOpType
    AX = mybir.AxisListType

    nc = bacc.Bacc("TRN2", target_bir_lowering=False, debug=False)

    xb = nc.dram_tensor("xb", [C, N], f32r, kind="ExternalInput").ap()
    wqt = nc.dram_tensor("wqt", [C, HC], f32r, kind="ExternalInput").ap()
    wkt = nc.dram_tensor("wkt", [C, HC], f32r, kind="ExternalInput").ap()
    wvt = nc.dram_tensor("wvt", [C, HC], f32r, kind="ExternalInput").ap()
    wot = nc.dram_tensor("wot", [HC, C], f32r, kind="ExternalInput").ap()
    bqh = nc.dram_tensor("bqh", [HC, 1], f32, kind="ExternalInput").ap()
    bkh = nc.dram_tensor("bkh", [HC, 1], f32, kind="ExternalInput").ap()
    bvh = nc.dram_tensor("bvh", [HC, 1], f32, kind="ExternalInput").ap()
    gns = nc.dram_tensor("gns", [1, C], f32, kind="ExternalInput").ap()
    gnb = nc.dram_tensor("gnb", [1, C], f32, kind="ExternalInput").ap()
    yp = nc.dram_tensor("yp", [C, N], f32, kind="ExternalOutput").ap()

    xbv = xb.rearrange("(a p) n -> a p n", p=P)            # [4, 128, 4096]
    wqv = wqt.rearrange("(a p) o -> p a o", p=P)           # [128, 4, 128]
    wkv = wkt.rearrange("(a p) o -> p a o", p=P)
    wvv = wvt.rearrange("(a p) o -> p a o", p=P)
    ypv = yp.rearrange("(oc p) (g i) -> oc p g i", p=P, i=IG)  # [4, 128, 8, 512]

    with tile.TileContext(nc) as tc, ExitStack() as ctx:
        consts = ctx.enter_context(tc.tile_pool(name="consts", bufs=1))
        qkv = ctx.enter_context(tc.tile_pool(name="qkv", bufs=1))
        small = ctx.enter_context(tc.tile_pool(name="small", bufs=2))
        otp = ctx.enter_context(tc.tile_pool(name="otp", bufs=2))
        yfp = ctx.enter_context(tc.tile_pool(name="yfp", bufs=2))
        bcp = ctx.enter_context(tc.tile_pool(name="bcp", bufs=2))
        pps = ctx.enter_context(tc.tile_pool(name="pps", bufs=2, space="PSUM"))

        # prologue-scoped pools (space reclaimed before the attention pools open)
        pro = ExitStack()
        xpool = pro.enter_context(tc.tile_pool(name="xpool", bufs=1))
        stats = pro.enter_context(tc.tile_pool(name="stats", bufs=1))
        stats2 = pro.enter_context(tc.tile_pool(name="stats2", bufs=2))
        ppt = pro.enter_context(tc.tile_pool(name="ppt", bufs=2, space="PSUM"))
        ppsm = pro.enter_context(tc.tile_pool(name="ppsm", bufs=2, space="PSUM"))

        # ---- constants / weights ----
        ident = consts.tile([P, P], f32)
        make_identity(nc, ident)
        ones_mat = consts.tile([P, P], f32)
        nc.vector.memset(ones_mat, 1.0)
        ones_r = consts.tile([P, P], f32r)
        nc.vector.tensor_copy(out=ones_r, in_=ones_mat)
        eps4 = consts.tile([NCH, 1], f32)
        nc.vector.memset(eps4, EPS)

        w_q = consts.tile([P, NCH, HC], f32r)
        nc.sync.dma_start(out=w_q, in_=wqv)
        w_k = consts.tile([P, NCH, HC], f32r)
        nc.sync.dma_start(out=w_k, in_=wkv)
        w_v = consts.tile([P, NCH, HC], f32r)
        nc.sync.dma_start(out=w_v, in_=wvv)
        w_o = consts.tile([P, C], f32r)
        nc.sync.dma_start(out=w_o, in_=wot)
        bq_sb = consts.tile([P, 1], f32)
        nc.sync.dma_start(out=bq_sb, in_=bqh)
        bk_sb = consts.tile([P, 1], f32)
        nc.sync.dma_start(out=bk_sb, in_=bkh)
        bv_sb = consts.tile([P, 1], f32)
        nc.sync.dma_start(out=bv_sb, in_=bvh)
        gns_sb = consts.tile([NCH, P], f32)
        nc.sync.dma_start(out=gns_sb, in_=gns.rearrange("a (b c) -> (a b) c", b=NCH))
        gnb_sb = consts.tile([NCH, P], f32)
        nc.sync.dma_start(out=gnb_sb, in_=gnb.rearrange("a (b c) -> (a b) c", b=NCH))

        # ---- load x ----
        # 16 slices per channel chunk: each chunk's transfers occupy all DMA
        # queues, so chunk ci completes at ~(ci+1)/4 of the total load time and
        # bn_stats can chase the DMA instead of waiting for the whole tensor.
        xc = xpool.tile([P, NCH, N], f32r)
        NSL = N // 16
        for ci in range(NCH):
            for sl in range(16):
                nc.sync.dma_start(
                    out=xc[:, ci, sl * NSL : (sl + 1) * NSL],
                    in_=xbv[ci][:, sl * NSL : (sl + 1) * NSL],
                )

        # ---- GroupNorm statistics (per-channel mean/var via bn_stats) ----
        mv = stats.tile([P, NCH, 2], f32)
        for ci in range(NCH):
            st = stats2.tile([P, 8, 6], f32)
            xv = xc[:, ci, :].bitcast(f32).rearrange("p (s f) -> p s f", f=512)
            for s in range(8):
                nc.vector.bn_stats(out=st[:, s, :], in_=xv[:, s, :])
            nc.vector.bn_aggr(out=mv[:, ci, :], in_=st)

        # vpm = var + mean^2 per channel  ([128, 4] wide ops)
        vpm = stats.tile([P, NCH], f32)
        nc.vector.tensor_mul(vpm, mv[:, :, 0], mv[:, :, 0])
        nc.vector.tensor_add(vpm, vpm, mv[:, :, 1])

        # transpose stats to chunk-major rows [4, 128] (one transpose each)
        mrow4 = stats.tile([NCH, P], f32)
        vrow4 = stats.tile([NCH, P], f32)
        pm = ppsm.tile([NCH, P], f32, tag="sm")
        nc.tensor.transpose(pm, mv[:, :, 0], ident)
        nc.vector.tensor_copy(out=mrow4, in_=pm)
        pv2 = ppsm.tile([NCH, P], f32, tag="sm")
        nc.tensor.transpose(pv2, vpm, ident)
        nc.vector.tensor_copy(out=vrow4, in_=pv2)

        # group-wise means over each run of 16 channels -> [4, 8]
        gm4 = stats.tile([NCH, 8], f32)
        gv4 = stats.tile([NCH, 8], f32)
        nc.vector.reduce_sum(
            out=gm4[:], in_=mrow4[:].rearrange("p (g s) -> p g s", s=GSIZE), axis=AX.X
        )
        nc.vector.tensor_scalar_mul(gm4, gm4, 1.0 / GSIZE)
        nc.vector.reduce_sum(
            out=gv4[:], in_=vrow4[:].rearrange("p (g s) -> p g s", s=GSIZE), axis=AX.X
        )
        nc.vector.tensor_scalar_mul(gv4, gv4, 1.0 / GSIZE)
        gm2 = stats.tile([NCH, 8], f32)
        nc.vector.tensor_mul(gm2, gm4, gm4)
        nc.vector.tensor_sub(gv4, gv4, gm2)        # gv4 = group variance
        nc.scalar.activation(out=gv4, in_=gv4, func=AF.Sqrt, bias=eps4)
        nc.vector.reciprocal(gv4, gv4)             # gv4 = rstd per group

        # expand groups to channels: [4, 8] -> [4, 128]
        grx = stats.tile([NCH, P], f32)
        gmx = stats.tile([NCH, P], f32)
        gv_ap = gv4[:]
        gm_ap = gm4[:]
        gv_b = bass.AP(tensor=gv_ap.tensor, offset=gv_ap.offset, ap=list(gv_ap.ap) + [[0, GSIZE]])
        gm_b = bass.AP(tensor=gm_ap.tensor, offset=gm_ap.offset, ap=list(gm_ap.ap) + [[0, GSIZE]])
        nc.vector.tensor_copy(out=grx[:].rearrange("p (g s) -> p g s", s=GSIZE), in_=gv_b)
        nc.vector.tensor_copy(out=gmx[:].rearrange("p (g s) -> p g s", s=GSIZE), in_=gm_b)

        # arow = rstd * gn_scale ; brow = gn_bias - mean * arow    ([4, 128] ops)
        nc.vector.tensor_mul(grx, grx, gns_sb)
        nc.vector.tensor_mul(gmx, gmx, grx)
        nc.vector.tensor_sub(gmx, gnb_sb, gmx)

        # transpose back to per-partition scalars [128, 4]
        acol = stats.tile([P, NCH], f32)
        bcol = stats.tile([P, NCH], f32)
        pa = ppsm.tile([P, NCH], f32, tag="sm")
        nc.tensor.transpose(pa, grx, ident[0:NCH, 0:NCH])
        nc.vector.tensor_copy(out=acol, in_=pa)
        pb = ppsm.tile([P, NCH], f32, tag="sm")
        nc.tensor.transpose(pb, gmx, ident[0:NCH, 0:NCH])
        nc.vector.tensor_copy(out=bcol, in_=pb)

        # apply GN in place: h_ = x * A + B  (chunks split across DVE and ACT)
        for ci in range(NCH):
            if ci % 2 == 0:
                nc.vector.tensor_scalar(
                    out=xc[:, ci, :],
                    in0=xc[:, ci, :].bitcast(f32),
                    scalar1=acol[:, ci : ci + 1],
                    scalar2=bcol[:, ci : ci + 1],
                    op0=OP.mult,
                    op1=OP.add,
                )
            else:
                nc.scalar.activation(
                    out=xc[:, ci, :],
                    in_=xc[:, ci, :].bitcast(f32),
                    func=AF.Identity,
                    bias=bcol[:, ci : ci + 1],
                    scale=acol[:, ci : ci + 1],
                )

        # ---- projections q, k, v ----
        q_sb = qkv.tile([P, N], f32r)
        k_sb = qkv.tile([P, N], f32r)
        v_sb = xpool.tile([P, N], f32)
        vt_sb = qkv.tile([P, NJT, HC], f32r)

        for w_sb, b_sb, dst in ((w_q, bq_sb, q_sb), (w_k, bk_sb, k_sb), (w_v, bv_sb, v_sb)):
            for nh in range(NIG):
                pp = pps.tile([P, IG], f32, tag="ps")
                for ci in range(NCH):
                    nc.tensor.matmul(
                        pp,
                        lhsT=w_sb[:, ci, :],
                        rhs=xc[:, ci, nh * IG : (nh + 1) * IG],
                        start=(ci == 0),
                        stop=(ci == NCH - 1),
                    )
                nc.scalar.activation(
                    out=dst[:, nh * IG : (nh + 1) * IG],
                    in_=pp,
                    func=AF.Identity,
                    bias=b_sb,
                    scale=1.0,
                )

        for jt in range(NJT):
            ptr = ppt.tile([P, P], f32)
            nc.tensor.transpose(ptr, v_sb[:, jt * P : (jt + 1) * P], ident)
            nc.vector.tensor_copy(out=vt_sb[:, jt, :], in_=ptr)

        pro.close()

        # attention-phase pools
        ptp = ctx.enter_context(tc.tile_pool(name="ptp", bufs=1))
        ppden = ctx.enter_context(tc.tile_pool(name="ppden", bufs=1, space="PSUM"))
        ppo = ctx.enter_context(tc.tile_pool(name="ppo", bufs=2, space="PSUM"))
        ppf = ctx.enter_context(tc.tile_pool(name="ppf", bufs=1, space="PSUM"))

        # ---- attention ----
        pt_big = ptp.tile([P, NJT, IG], f32r)
        for g in range(NIG):
            qs = q_sb[:, g * IG : (g + 1) * IG]

            # S^T chunk-pair matmuls + one exp per 1024 columns, then a
            # pair-sum on DVE/GpSimd so the denominator matmul only needs
            # 16 chunks
            for jp in range(NJT // 2):
                ps = pps.tile([P, 2, IG], f32, tag="ps")
                for h in range(2):
                    jt = 2 * jp + h
                    nc.tensor.matmul(
                        ps[:, h, :],
                        lhsT=k_sb[:, jt * P : (jt + 1) * P],
                        rhs=qs,
                        start=True,
                        stop=True,
                    )
                nc.scalar.activation(
                    out=pt_big[:, 2 * jp : 2 * jp + 2, :],
                    in_=ps,
                    func=AF.Exp,
                    scale=SCALE,
                )

            # denominators: ones-matrix matmul -> every partition holds the sums
            pden = ppden.tile([P, IG], f32)
            for jt in range(NJT):
                nc.tensor.matmul(
                    pden,
                    lhsT=ones_r,
                    rhs=pt_big[:, jt, :],
                    start=(jt == 0),
                    stop=(jt == NJT - 1),
                )

            po = ppo.tile([P, IG], f32)
            for jt in range(NJT):
                nc.tensor.matmul(
                    po,
                    lhsT=vt_sb[:, jt, :],
                    rhs=pt_big[:, jt, :],
                    start=(jt == 0),
                    stop=(jt == NJT - 1),
                )

            bc = bcp.tile([P, IG], f32)
            nc.vector.reciprocal(bc, pden)
            ot = otp.tile([P, IG], f32r)
            nc.vector.tensor_mul(ot, po, bc)

            for oc in range(NCH):
                pf = ppf.tile([P, IG], f32)
                nc.tensor.matmul(pf, lhsT=w_o[:, oc * P : (oc + 1) * P], rhs=ot, start=True, stop=True)
                yf = yfp.tile([P, IG], f32)
                nc.vector.tensor_copy(out=yf, in_=pf)
                nc.sync.dma_start(out=ypv[oc, :, g, :], in_=yf)

    nc.compile()
    return nc


def get_nc():
    if "nc" not in _NC_CACHE:
        _NC_CACHE["nc"] = _build_nc()
    return _NC_CACHE["nc"]


def make_in_maps(inputs):
    x = np.ascontiguousarray(np.asarray(inputs["x"], dtype=np.float32))
    wq = np.asarray(inputs["wq"], np.float32)
    wk = np.asarray(inputs["wk"], np.float32)
    wv = np.asarray(inputs["wv"], np.float32)
    bq = np.asarray(inputs["bq"], np.float32)
    bk = np.asarray(inputs["bk"], np.float32)
    bv = np.asarray(inputs["bv"], np.float32)
    wo = np.asarray(inputs["wo"], np.float32)
    gn_scale = np.asarray(inputs["gn_scale"], np.float32)
    gn_bias = np.asarray(inputs["gn_bias"], np.float32)

    in_maps = []
    for cid in range(8):
        b, h = divmod(cid, HEADS)
        sl = slice(h * HC, (h + 1) * HC)
        in_maps.append(
            {
                "xb": x[b].reshape(C, N),
                "wqt": np.ascontiguousarray(wq[sl, :].T),
                "wkt": np.ascontiguousarray(wk[sl, :].T),
                "wvt": np.ascontiguousarray(wv[sl, :].T),
                "wot": np.ascontiguousarray(wo[:, sl].T),
                "bqh": np.ascontiguousarray(bq[sl].reshape(HC, 1)),
                "bkh": np.ascontiguousarray(bk[sl].reshape(HC, 1)),
                "bvh": np.ascontiguousarray(bv[sl].reshape(HC, 1)),
                "gns": np.ascontiguousarray(gn_scale.reshape(1, C)),
                "gnb": np.ascontiguousarray(gn_bias.reshape(1, C)),
            }
        )
    return in_maps


def assemble_output(inputs, yps):
    x = np.asarray(inputs["x"], np.float32)
    bo = np.asarray(inputs["bo"], np.float32)
    y = x.reshape(2, C, N).astype(np.float32).copy()
    y += bo.reshape(1, C, 1)
    for cid in range(8):
        b = cid // HEADS
        y[b] += yps[cid]
    return y.reshape(2, C, 64, 64)


def run(inputs, trace=False):
    from concourse.bass_utils import run_bass_kernel_spmd

    nc = get_nc()
    in_maps = make_in_maps(inputs)
    res = run_bass_kernel_spmd(nc, in_maps, list(range(8)), trace=trace)
    yps = [r["yp"] for r in res.results]
    return assemble_output(inputs, yps), res


def kernel(**inputs):
    y, _ = run(inputs, trace=False)
    return y


# revision 20
# speedup vs baseline: 1.0614x; 1.0614x over previous
"""Trainium2 Bass kernel for nn_AttnBlock (GroupNorm + single-head-split attention + residual).

Sharding: 8 cores = (batch b in {0,1}) x (head h in {0..3}).  Each core computes
the full attention for its (b, h) pair plus the partial output projection
wo[:, head_cols] @ att_out_head -> [512, 4096].  Host sums the 4 head partials
per batch, adds the residual x and output bias bo.

Per-core kernel (all fp32):
  1. GroupNorm(32 groups) of x[b] [512, 4096]  (channels on partitions, 4 chunks of 128)
  2. q = wq_h @ h_, k = wk_h @ h_, v = wv_h @ h_   ([128, 4096] each; head slices only)
  3. vT via PE transposes ([4096, 128] as 32 tiles)
  4. For each 512-query group: S^T[j,i] = k^T q (32 j-tiles), P = exp(scale*S^T) (ACT),
     denominators via ones-matmul, out^T = V P (PSUM accumulation), normalize with
     gpsimd partition_broadcast of 1/den, final partial = wo_h^T @ out^T.
"""

import sys

sys.path.insert(0, "/opt/trn_rl_repo")

import numpy as np

C = 512
HEADS = 4
HC = 128          # head channels
N = 4096          # h*w pixels
P = 128           # partitions
NCH = C // P      # 4 channel chunks
NJT = N // P      # 32 key tiles
IG = 512          # query-group width
NIG = N // IG     # 8 query groups
GSIZE = 16        # channels per groupnorm group
NGRP = 32
EPS = 1e-6
SCALE = float(C) ** -0.5
DEN_LAG = 3       # delay (in j-tiles) of the denominator matmul behind the S^T matmul

_NC_CACHE = {}


def _build_nc():
    from contextlib import ExitStack

    import concourse.bacc as bacc
    import concourse.bass as bass
    import concourse.tile as tile
    from concourse import mybir
    from concourse.masks import make_identity

    f32 = mybir.dt.float32
    f32r = mybir.dt.float32r

    def r(ap):
        return ap.bitcast(f32r)
    AF = mybir.ActivationFunctionType
    OP = mybir.AluOpType
    AX = mybir.AxisListType

    nc = bacc.Bacc("TRN2", target_bir_lowering=False, debug=False)

    xb = nc.dram_tensor("xb", [C, N], f32r, kind="ExternalInput").ap()
    wqt = nc.dram_tensor("wqt", [C, HC], f32r, kind="ExternalInput").ap()
    wkt = nc.dram_tensor("wkt", [C, HC], f32r, kind="ExternalInput").ap()
    wvt = nc.dram_tensor("wvt", [C, HC], f32r, kind="ExternalInput").ap()
    wot = nc.dram_tensor("wot", [HC, C], f32r, kind="ExternalInput").ap()
    bqh = nc.dram_tensor("bqh", [HC, 1], f32, kind="ExternalInput").ap()
    bkh = nc.dram_tensor("bkh", [HC, 1], f32, kind="ExternalInput").ap()
    bvh = nc.dram_tensor("bvh", [HC, 1], f32, kind="ExternalInput").ap()
    gns = nc.dram_tensor("gns", [1, C], f32, kind="ExternalInput").ap()
    gnb = nc.dram_tensor("gnb", [1, C], f32, kind="ExternalInput").ap()
    yp = nc.dram_tensor("yp", [C, N], f32, kind="ExternalOutput").ap()

    xbv = xb.rearrange("(a p) n -> a p n", p=P)            # [4, 128, 4096]
    wqv = wqt.rearrange("(a p) o -> p a o", p=P)           # [128, 4, 128]
    wkv = wkt.rearrange("(a p) o -> p a o", p=P)
    wvv = wvt.rearrange("(a p) o -> p a o", p=P)
    ypv = yp.rearrange("(oc p) (g i) -> oc p g i", p=P, i=IG)  # [4, 128, 8, 512]

    with tile.TileContext(nc) as tc, ExitStack() as ctx:
        consts = ctx.enter_context(tc.tile_pool(name="consts", bufs=1))
        qkv = ctx.enter_context(tc.tile_pool(name="qkv", bufs=1))
        small = ctx.enter_context(tc.tile_pool(name="small", bufs=2))
        otp = ctx.enter_context(tc.tile_pool(name="otp", bufs=2))
        yfp = ctx.enter_context(tc.tile_pool(name="yfp", bufs=2))
        bcp = ctx.enter_context(tc.tile_pool(name="bcp", bufs=2))
        pps = ctx.enter_context(tc.tile_pool(name="pps", bufs=2, space="PSUM"))

        # prologue-scoped pools (space reclaimed before the attention pools open)
        pro = ExitStack()
        xpool = pro.enter_context(tc.tile_pool(name="xpool", bufs=1))
        stats = pro.enter_context(tc.tile_pool(name="stats", bufs=1))
        stats2 = pro.enter_context(tc.tile_pool(name="stats2", bufs=2))
        ppt = pro.enter_context(tc.tile_pool(name="ppt", bufs=2, space="PSUM"))
        ppsm = pro.enter_context(tc.tile_pool(name="ppsm", bufs=2, space="PSUM"))

        # ---- constants / weights ----
        ident = consts.tile([P, P], f32)
        make_identity(nc, ident)
        ones_mat = consts.tile([P, P], f32)
        nc.vector.memset(ones_mat, 1.0)
        ones_r = consts.tile([P, P], f32r)
        nc.vector.tensor_copy(out=ones_r, in_=ones_mat)
        eps4 = consts.tile([NCH, 1], f32)
        nc.vector.memset(eps4, EPS)

        w_q = consts.tile([P, NCH, HC], f32r)
        nc.sync.dma_start(out=w_q, in_=wqv)
        w_k = consts.tile([P, NCH, HC], f32r)
        nc.sync.dma_start(out=w_k, in_=wkv)
        w_v = consts.tile([P, NCH, HC], f32r)
        nc.sync.dma_start(out=w_v, in_=wvv)
        w_o = consts.tile([P, C], f32r)
        nc.sync.dma_start(out=w_o, in_=wot)
        bq_sb = consts.tile([P, 1], f32)
        nc.sync.dma_start(out=bq_sb, in_=bqh)
        bk_sb = consts.tile([P, 1], f32)
        nc.sync.dma_start(out=bk_sb, in_=bkh)
        bv_sb = consts.tile([P, 1], f32)
        nc.sync.dma_start(out=bv_sb, in_=bvh)
        gns_sb = consts.tile([NCH, P], f32)
        nc.sync.dma_start(out=gns_sb, in_=gns.rearrange("a (b c) -> (a b) c", b=NCH))
        gnb_sb = consts.tile([NCH, P], f32)
        nc.sync.dma_start(out=gnb_sb, in_=gnb.rearrange("a (b c) -> (a b) c", b=NCH))

        # ---- load x ----
        # 16 slices per channel chunk: each chunk's transfers occupy all DMA
        # queues, so chunk ci completes at ~(ci+1)/4 of the total load time and
        # bn_stats can chase the DMA instead of waiting for the whole tensor.
        xc = xpool.tile([P, NCH, N], f32r)
        NSL = N // 8
        for ci in range(NCH):
            for sl in range(8):
                nc.sync.dma_start(
                    out=xc[:, ci, sl * NSL : (sl + 1) * NSL],
                    in_=xbv[ci][:, sl * NSL : (sl + 1) * NSL],
                )

        # ---- GroupNorm statistics (per-channel mean/var via bn_stats) ----
        mv = stats.tile([P, NCH, 2], f32)
        for ci in range(NCH):
            st = stats2.tile([P, 8, 6], f32)
            xv = xc[:, ci, :].bitcast(f32).rearrange("p (s f) -> p s f", f=512)
            for s in range(8):
                nc.vector.bn_stats(out=st[:, s, :], in_=xv[:, s, :])
            nc.vector.bn_aggr(out=mv[:, ci, :], in_=st)

        # vpm = var + mean^2 per channel  ([128, 4] wide ops)
        vpm = stats.tile([P, NCH], f32)
        nc.vector.tensor_mul(vpm, mv[:, :, 0], mv[:, :, 0])
        nc.vector.tensor_add(vpm, vpm, mv[:, :, 1])

        # transpose stats to chunk-major rows [4, 128] (one transpose each)
        mrow4 = stats.tile([NCH, P], f32)
        vrow4 = stats.tile([NCH, P], f32)
        pm = ppsm.tile([NCH, P], f32, tag="sm")
        nc.tensor.transpose(pm, mv[:, :, 0], ident)
        nc.vector.tensor_copy(out=mrow4, in_=pm)
        pv2 = ppsm.tile([NCH, P], f32, tag="sm")
        nc.tensor.transpose(pv2, vpm, ident)
        nc.vector.tensor_copy(out=vrow4, in_=pv2)

        # group-wise means over each run of 16 channels -> [4, 8]
        gm4 = stats.tile([NCH, 8], f32)
        gv4 = stats.tile([NCH, 8], f32)
        nc.vector.reduce_sum(
            out=gm4[:], in_=mrow4[:].rearrange("p (g s) -> p g s", s=GSIZE), axis=AX.X
        )
        nc.vector.tensor_scalar_mul(gm4, gm4, 1.0 / GSIZE)
        nc.vector.reduce_sum(
            out=gv4[:], in_=vrow4[:].rearrange("p (g s) -> p g s", s=GSIZE), axis=AX.X
        )
        nc.vector.tensor_scalar_mul(gv4, gv4, 1.0 / GSIZE)
        gm2 = stats.tile([NCH, 8], f32)
        nc.vector.tensor_mul(gm2, gm4, gm4)
        nc.vector.tensor_sub(gv4, gv4, gm2)        # gv4 = group variance
        nc.scalar.activation(out=gv4, in_=gv4, func=AF.Sqrt, bias=eps4)
        nc.vector.reciprocal(gv4, gv4)             # gv4 = rstd per group

        # expand groups to channels: [4, 8] -> [4, 128]
        grx = stats.tile([NCH, P], f32)
        gmx = stats.tile([NCH, P], f32)
        gv_ap = gv4[:]
        gm_ap = gm4[:]
        gv_b = bass.AP(tensor=gv_ap.tensor, offset=gv_ap.offset, ap=list(gv_ap.ap) + [[0, GSIZE]])
        gm_b = bass.AP(tensor=gm_ap.tensor, offset=gm_ap.offset, ap=list(gm_ap.ap) + [[0, GSIZE]])
        nc.vector.tensor_copy(out=grx[:].rearrange("p (g s) -> p g s", s=GSIZE), in_=gv_b)
        nc.vector.tensor_copy(out=gmx[:].rearrange("p (g s) -> p g s", s=GSIZE), in_=gm_b)

        # arow = rstd * gn_scale ; brow = gn_bias - mean * arow    ([4, 128] ops)
        nc.vector.tensor_mul(grx, grx, gns_sb)
        nc.vector.tensor_mul(gmx, gmx, grx)
        nc.vector.tensor_sub(gmx, gnb_sb, gmx)

        # transpose back to per-partition scalars [128, 4]
        acol = stats.tile([P, NCH], f32)
        bcol = stats.tile([P, NCH], f32)
        pa = ppsm.tile([P, NCH], f32, tag="sm")
        nc.tensor.transpose(pa, grx, ident[0:NCH, 0:NCH])
        nc.vector.tensor_copy(out=acol, in_=pa)
        pb = ppsm.tile([P, NCH], f32, tag="sm")
        nc.tensor.transpose(pb, gmx, ident[0:NCH, 0:NCH])
        nc.vector.tensor_copy(out=bcol, in_=pb)

        # apply GN in place: h_ = x * A + B  (chunks split across DVE and ACT)
        for ci in range(NCH):
            if ci % 2 == 0:
                nc.vector.tensor_scalar(
                    out=xc[:, ci, :],
                    in0=xc[:, ci, :].bitcast(f32),
                    scalar1=acol[:, ci : ci + 1],
                    scalar2=bcol[:, ci : ci + 1],
                    op0=OP.mult,
                    op1=OP.add,
                )
            else:
                nc.scalar.activation(
                    out=xc[:, ci, :],
                    in_=xc[:, ci, :].bitcast(f32),
                    func=AF.Identity,
                    bias=bcol[:, ci : ci + 1],
                    scale=acol[:, ci : ci + 1],
                )

        # ---- projections q, k, v ----
        q_sb = qkv.tile([P, N], f32r)
        k_sb = qkv.tile([P, N], f32r)
        v_sb = xpool.tile([P, N], f32)
        vt_sb = qkv.tile([P, NJT, HC], f32r)

        for w_sb, b_sb, dst in ((w_q, bq_sb, q_sb), (w_k, bk_sb, k_sb), (w_v, bv_sb, v_sb)):
            for nh in range(NIG):
                pp = pps.tile([P, IG], f32, tag="ps")
                for ci in range(NCH):
                    nc.tensor.matmul(
                        pp,
                        lhsT=w_sb[:, ci, :],
                        rhs=xc[:, ci, nh * IG : (nh + 1) * IG],
                        start=(ci == 0),
                        stop=(ci == NCH - 1),
                    )
                nc.scalar.activation(
                    out=dst[:, nh * IG : (nh + 1) * IG],
                    in_=pp,
                    func=AF.Identity,
                    bias=b_sb,
                    scale=1.0,
                )

        for jt in range(NJT):
            ptr = ppt.tile([P, P], f32)
            nc.tensor.transpose(ptr, v_sb[:, jt * P : (jt + 1) * P], ident)
            nc.vector.tensor_copy(out=vt_sb[:, jt, :], in_=ptr)

        pro.close()

        # attention-phase pools
        ptp = ctx.enter_context(tc.tile_pool(name="ptp", bufs=1))
        ppden = ctx.enter_context(tc.tile_pool(name="ppden", bufs=1, space="PSUM"))
        ppo = ctx.enter_context(tc.tile_pool(name="ppo", bufs=2, space="PSUM"))
        ppf = ctx.enter_context(tc.tile_pool(name="ppf", bufs=1, space="PSUM"))

        # ---- attention ----
        pt_big = ptp.tile([P, NJT, IG], f32r)
        for g in range(NIG):
            qs = q_sb[:, g * IG : (g + 1) * IG]

            # S^T chunk-pair matmuls + one exp per 1024 columns, then a
            # pair-sum on DVE/GpSimd so the denominator matmul only needs
            # 16 chunks
            for jp in range(NJT // 2):
                ps = pps.tile([P, 2, IG], f32, tag="ps")
                for h in range(2):
                    jt = 2 * jp + h
                    nc.tensor.matmul(
                        ps[:, h, :],
                        lhsT=k_sb[:, jt * P : (jt + 1) * P],
                        rhs=qs,
                        start=True,
                        stop=True,
                    )
                nc.scalar.activation(
                    out=pt_big[:, 2 * jp : 2 * jp + 2, :],
                    in_=ps,
                    func=AF.Exp,
                    scale=SCALE,
                )

            # denominators: ones-matrix matmul -> every partition holds the sums
            pden = ppden.tile([P, IG], f32)
            for jt in range(NJT):
                nc.tensor.matmul(
                    pden,
                    lhsT=ones_r,
                    rhs=pt_big[:, jt, :],
                    start=(jt == 0),
                    stop=(jt == NJT - 1),
                )

            po = ppo.tile([P, IG], f32)
            for jt in range(NJT):
                nc.tensor.matmul(
                    po,
                    lhsT=vt_sb[:, jt, :],
                    rhs=pt_big[:, jt, :],
                    start=(jt == 0),
                    stop=(jt == NJT - 1),
                )

            bc = bcp.tile([P, IG], f32)
            nc.vector.reciprocal(bc, pden)
            ot = otp.tile([P, IG], f32r)
            nc.vector.tensor_mul(ot, po, bc)

            for oc in range(NCH):
                pf = ppf.tile([P, IG], f32)
                nc.tensor.matmul(pf, lhsT=w_o[:, oc * P : (oc + 1) * P], rhs=ot, start=True, stop=True)
                yf = yfp.tile([P, IG], f32)
                nc.vector.tensor_copy(out=yf, in_=pf)
                nc.sync.dma_start(out=ypv[oc, :, g, :], in_=yf)

    nc.compile()
    return nc


def get_nc():
    if "nc" not in _NC_CACHE:
        _NC_CACHE["nc"] = _build_nc()
    return _NC_CACHE["nc"]


def make_in_maps(inputs):
    x = np.ascontiguousarray(np.asarray(inputs["x"], dtype=np.float32))
    wq = np.asarray(inputs["wq"], np.float32)
    wk = np.asarray(inputs["wk"], np.float32)
    wv = np.asarray(inputs["wv"], np.float32)
    bq = np.asarray(inputs["bq"], np.float32)
    bk = np.asarray(inputs["bk"], np.float32)
    bv = np.asarray(inputs["bv"], np.float32)
    wo = np.asarray(inputs["wo"], np.float32)
    gn_scale = np.asarray(inputs["gn_scale"], np.float32)
    gn_bias = np.asarray(inputs["gn_bias"], np.float32)

    in_maps = []
    for cid in range(8):
        b, h = divmod(cid, HEADS)
        sl = slice(h * HC, (h + 1) * HC)
        in_maps.append(
            {
                "xb": x[b].reshape(C, N),
                "wqt": np.ascontiguousarray(wq[sl, :].T),
                "wkt": np.ascontiguousarray(wk[sl, :].T),
                "wvt": np.ascontiguousarray(wv[sl, :].T),
                "wot": np.ascontiguousarray(wo[:, sl].T),
                "bqh": np.ascontiguousarray(bq[sl].reshape(HC, 1)),
                "bkh": np.ascontiguousarray(bk[sl].reshape(HC, 1)),
                "bvh": np.ascontiguousarray(bv[sl].reshape(HC, 1)),
                "gns": np.ascontiguousarray(gn_scale.reshape(1, C)),
                "gnb": np.ascontiguousarray(gn_bias.reshape(1, C)),
            }
        )
    return in_maps


def assemble_output(inputs, yps):
    x = np.asarray(inputs["x"], np.float32)
    bo = np.asarray(inputs["bo"], np.float32)
    y = x.reshape(2, C, N).astype(np.float32).copy()
    y += bo.reshape(1, C, 1)
    for cid in range(8):
        b = cid // HEADS
        y[b] += yps[cid]
    return y.reshape(2, C, 64, 64)


def run(inputs, trace=False):
    from concourse.bass_utils import run_bass_kernel_spmd

    nc = get_nc()
    in_maps = make_in_maps(inputs)
    res = run_bass_kernel_spmd(nc, in_maps, list(range(8)), trace=trace)
    yps = [r["yp"] for r in res.results]
    return assemble_output(inputs, yps), res


def kernel(**inputs):
    y, _ = run(inputs, trace=False)
    return y


# revision 21
# speedup vs baseline: 1.0792x; 1.0167x over previous
"""Trainium2 Bass kernel for nn_AttnBlock (GroupNorm + single-head-split attention + residual).

Sharding: 8 cores = (batch b in {0,1}) x (head h in {0..3}).  Each core computes
the full attention for its (b, h) pair plus the partial output projection
wo[:, head_cols] @ att_out_head -> [512, 4096].  Host sums the 4 head partials
per batch, adds the residual x and output bias bo.

Per-core kernel (all fp32):
  1. GroupNorm(32 groups) of x[b] [512, 4096]  (channels on partitions, 4 chunks of 128)
  2. q = wq_h @ h_, k = wk_h @ h_, v = wv_h @ h_   ([128, 4096] each; head slices only)
  3. vT via PE transposes ([4096, 128] as 32 tiles)
  4. For each 512-query group: S^T[j,i] = k^T q (32 j-tiles), P = exp(scale*S^T) (ACT),
     denominators via ones-matmul, out^T = V P (PSUM accumulation), normalize with
     gpsimd partition_broadcast of 1/den, final partial = wo_h^T @ out^T.
"""

import sys

sys.path.insert(0, "/opt/trn_rl_repo")

import numpy as np

C = 512
HEADS = 4
HC = 128          # head channels
N = 4096          # h*w pixels
P = 128           # partitions
NCH = C // P      # 4 channel chunks
NJT = N // P      # 32 key tiles
IG = 512          # query-group width
NIG = N // IG     # 8 query groups
GSIZE = 16        # channels per groupnorm group
NGRP = 32
EPS = 1e-6
SCALE = float(C) ** -0.5
DEN_LAG = 3       # delay (in j-tiles) of the denominator matmul behind the S^T matmul

_NC_CACHE = {}


def _build_nc():
    from contextlib import ExitStack

    import concourse.bacc as bacc
    import concourse.bass as bass
    import concourse.tile as tile
    from concourse import mybir
    from concourse.masks import make_identity

    f32 = mybir.dt.float32
    f32r = mybir.dt.float32r

    def r(ap):
        return ap.bitcast(f32r)
    AF = mybir.ActivationFunctionType
    OP = mybir.AluOpType
    AX = mybir.AxisListType

    nc = bacc.Bacc("TRN2", target_bir_lowering=False, debug=False)

    xb = nc.dram_tensor("xb", [C, N], f32r, kind="ExternalInput").ap()
    wqt = nc.dram_tensor("wqt", [C, HC], f32r, kind="ExternalInput").ap()
    wkt = nc.dram_tensor("wkt", [C, HC], f32r, kind="ExternalInput").ap()
    wvt = nc.dram_tensor("wvt", [C, HC], f32r, kind="ExternalInput").ap()
    wot = nc.dram_tensor("wot", [HC, C], f32r, kind="ExternalInput").ap()
    bqh = nc.dram_tensor("bqh", [HC, 1], f32, kind="ExternalInput").ap()
    bkh = nc.dram_tensor("bkh", [HC, 1], f32, kind="ExternalInput").ap()
    bvh = nc.dram_tensor("bvh", [HC, 1], f32, kind="ExternalInput").ap()
    gns = nc.dram_tensor("gns", [1, C], f32, kind="ExternalInput").ap()
    gnb = nc.dram_tensor("gnb", [1, C], f32, kind="ExternalInput").ap()
    yp = nc.dram_tensor("yp", [C, N], f32, kind="ExternalOutput").ap()

    xbv = xb.rearrange("(a p) n -> a p n", p=P)            # [4, 128, 4096]
    wqv = wqt.rearrange("(a p) o -> p a o", p=P)           # [128, 4, 128]
    wkv = wkt.rearrange("(a p) o -> p a o", p=P)
    wvv = wvt.rearrange("(a p) o -> p a o", p=P)
    ypv = yp.rearrange("(oc p) (g i) -> oc p g i", p=P, i=IG)  # [4, 128, 8, 512]

    with tile.TileContext(nc) as tc, ExitStack() as ctx:
        consts = ctx.enter_context(tc.tile_pool(name="consts", bufs=1))
        qkv = ctx.enter_context(tc.tile_pool(name="qkv", bufs=1))
        small = ctx.enter_context(tc.tile_pool(name="small", bufs=2))
        otp = ctx.enter_context(tc.tile_pool(name="otp", bufs=2))
        yfp = ctx.enter_context(tc.tile_pool(name="yfp", bufs=2))
        bcp = ctx.enter_context(tc.tile_pool(name="bcp", bufs=2))
        pps = ctx.enter_context(tc.tile_pool(name="pps", bufs=2, space="PSUM"))

        # prologue-scoped pools (space reclaimed before the attention pools open)
        pro = ExitStack()
        xpool = pro.enter_context(tc.tile_pool(name="xpool", bufs=1))
        stats = pro.enter_context(tc.tile_pool(name="stats", bufs=1))
        stats2 = pro.enter_context(tc.tile_pool(name="stats2", bufs=2))
        ppt = pro.enter_context(tc.tile_pool(name="ppt", bufs=2, space="PSUM"))
        ppsm = pro.enter_context(tc.tile_pool(name="ppsm", bufs=2, space="PSUM"))

        # ---- constants / weights ----
        ident = consts.tile([P, P], f32)
        make_identity(nc, ident)
        ones_mat = consts.tile([P, P], f32)
        nc.vector.memset(ones_mat, 1.0)
        ones_r = consts.tile([P, P], f32r)
        nc.vector.tensor_copy(out=ones_r, in_=ones_mat)
        eps4 = consts.tile([NCH, 1], f32)
        nc.vector.memset(eps4, EPS)

        # ---- load x first (critical path), 8 slices per channel chunk so a
        # chunk's statistics can start as soon as that chunk's queues drain ----
        xcs = [xpool.tile([P, N], f32r, name=f"xch{i}", tag=f"xch{i}") for i in range(NCH)]
        NSL = N // 8
        for ci in range(NCH):
            for sl in range(8):
                nc.sync.dma_start(
                    out=xcs[ci][:, sl * NSL : (sl + 1) * NSL],
                    in_=xbv[ci][:, sl * NSL : (sl + 1) * NSL],
                )

        w_q = consts.tile([P, NCH, HC], f32r)
        nc.sync.dma_start(out=w_q, in_=wqv)
        w_k = consts.tile([P, NCH, HC], f32r)
        nc.sync.dma_start(out=w_k, in_=wkv)
        w_v = consts.tile([P, NCH, HC], f32r)
        nc.sync.dma_start(out=w_v, in_=wvv)
        w_o = consts.tile([P, C], f32r)
        nc.sync.dma_start(out=w_o, in_=wot)
        bq_sb = consts.tile([P, 1], f32)
        nc.sync.dma_start(out=bq_sb, in_=bqh)
        bk_sb = consts.tile([P, 1], f32)
        nc.sync.dma_start(out=bk_sb, in_=bkh)
        bv_sb = consts.tile([P, 1], f32)
        nc.sync.dma_start(out=bv_sb, in_=bvh)
        gns_sb = consts.tile([NCH, P], f32)
        nc.sync.dma_start(out=gns_sb, in_=gns.rearrange("a (b c) -> (a b) c", b=NCH))
        gnb_sb = consts.tile([NCH, P], f32)
        nc.sync.dma_start(out=gnb_sb, in_=gnb.rearrange("a (b c) -> (a b) c", b=NCH))

        # ---- GroupNorm statistics (per-channel mean/var via bn_stats) ----
        mv = stats.tile([P, NCH, 2], f32)
        for ci in range(NCH):
            st = stats2.tile([P, 8, 6], f32)
            xv = xcs[ci][:].bitcast(f32).rearrange("p (s f) -> p s f", f=512)
            for s in range(8):
                nc.vector.bn_stats(out=st[:, s, :], in_=xv[:, s, :])
            nc.vector.bn_aggr(out=mv[:, ci, :], in_=st)

        # vpm = var + mean^2 per channel  ([128, 4] wide ops)
        vpm = stats.tile([P, NCH], f32)
        nc.vector.tensor_mul(vpm, mv[:, :, 0], mv[:, :, 0])
        nc.vector.tensor_add(vpm, vpm, mv[:, :, 1])

        # transpose stats to chunk-major rows [4, 128] (one transpose each)
        mrow4 = stats.tile([NCH, P], f32)
        vrow4 = stats.tile([NCH, P], f32)
        pm = ppsm.tile([NCH, P], f32, tag="sm")
        nc.tensor.transpose(pm, mv[:, :, 0], ident)
        nc.vector.tensor_copy(out=mrow4, in_=pm)
        pv2 = ppsm.tile([NCH, P], f32, tag="sm")
        nc.tensor.transpose(pv2, vpm, ident)
        nc.vector.tensor_copy(out=vrow4, in_=pv2)

        # group-wise means over each run of 16 channels -> [4, 8]
        gm4 = stats.tile([NCH, 8], f32)
        gv4 = stats.tile([NCH, 8], f32)
        nc.vector.reduce_sum(
            out=gm4[:], in_=mrow4[:].rearrange("p (g s) -> p g s", s=GSIZE), axis=AX.X
        )
        nc.vector.tensor_scalar_mul(gm4, gm4, 1.0 / GSIZE)
        nc.vector.reduce_sum(
            out=gv4[:], in_=vrow4[:].rearrange("p (g s) -> p g s", s=GSIZE), axis=AX.X
        )
        nc.vector.tensor_scalar_mul(gv4, gv4, 1.0 / GSIZE)
        gm2 = stats.tile([NCH, 8], f32)
        nc.vector.tensor_mul(gm2, gm4, gm4)
        nc.vector.tensor_sub(gv4, gv4, gm2)        # gv4 = group variance
        nc.scalar.activation(out=gv4, in_=gv4, func=AF.Sqrt, bias=eps4)
        nc.vector.reciprocal(gv4, gv4)             # gv4 = rstd per group

        # expand groups to channels: [4, 8] -> [4, 128]
        grx = stats.tile([NCH, P], f32)
        gmx = stats.tile([NCH, P], f32)
        gv_ap = gv4[:]
        gm_ap = gm4[:]
        gv_b = bass.AP(tensor=gv_ap.tensor, offset=gv_ap.offset, ap=list(gv_ap.ap) + [[0, GSIZE]])
        gm_b = bass.AP(tensor=gm_ap.tensor, offset=gm_ap.offset, ap=list(gm_ap.ap) + [[0, GSIZE]])
        nc.vector.tensor_copy(out=grx[:].rearrange("p (g s) -> p g s", s=GSIZE), in_=gv_b)
        nc.vector.tensor_copy(out=gmx[:].rearrange("p (g s) -> p g s", s=GSIZE), in_=gm_b)

        # arow = rstd * gn_scale ; brow = gn_bias - mean * arow    ([4, 128] ops)
        nc.vector.tensor_mul(grx, grx, gns_sb)
        nc.vector.tensor_mul(gmx, gmx, grx)
        nc.vector.tensor_sub(gmx, gnb_sb, gmx)

        # transpose back to per-partition scalars [128, 4]
        acol = stats.tile([P, NCH], f32)
        bcol = stats.tile([P, NCH], f32)
        pa = ppsm.tile([P, NCH], f32, tag="sm")
        nc.tensor.transpose(pa, grx, ident[0:NCH, 0:NCH])
        nc.vector.tensor_copy(out=acol, in_=pa)
        pb = ppsm.tile([P, NCH], f32, tag="sm")
        nc.tensor.transpose(pb, gmx, ident[0:NCH, 0:NCH])
        nc.vector.tensor_copy(out=bcol, in_=pb)

        # apply GN in place: h_ = x * A + B  (chunks split across DVE and ACT)
        for ci in range(NCH):
            if ci % 2 == 0:
                nc.vector.tensor_scalar(
                    out=xcs[ci][:],
                    in0=xcs[ci][:].bitcast(f32),
                    scalar1=acol[:, ci : ci + 1],
                    scalar2=bcol[:, ci : ci + 1],
                    op0=OP.mult,
                    op1=OP.add,
                )
            else:
                nc.scalar.activation(
                    out=xcs[ci][:],
                    in_=xcs[ci][:].bitcast(f32),
                    func=AF.Identity,
                    bias=bcol[:, ci : ci + 1],
                    scale=acol[:, ci : ci + 1],
                )

        # ---- projections q, k, v ----
        q_sb = qkv.tile([P, N], f32r)
        k_sb = qkv.tile([P, N], f32r)
        v_sb = xpool.tile([P, N], f32)
        vt_sb = qkv.tile([P, NJT, HC], f32r)

        for w_sb, b_sb, dst in ((w_q, bq_sb, q_sb), (w_k, bk_sb, k_sb), (w_v, bv_sb, v_sb)):
            for nh in range(NIG):
                pp = pps.tile([P, IG], f32, tag="ps")
                for ci in range(NCH):
                    nc.tensor.matmul(
                        pp,
                        lhsT=w_sb[:, ci, :],
                        rhs=xcs[ci][:, nh * IG : (nh + 1) * IG],
                        start=(ci == 0),
                        stop=(ci == NCH - 1),
                    )
                nc.scalar.activation(
                    out=dst[:, nh * IG : (nh + 1) * IG],
                    in_=pp,
                    func=AF.Identity,
                    bias=b_sb,
                    scale=1.0,
                )

        for jt in range(NJT):
            ptr = ppt.tile([P, P], f32)
            nc.tensor.transpose(ptr, v_sb[:, jt * P : (jt + 1) * P], ident)
            nc.vector.tensor_copy(out=vt_sb[:, jt, :], in_=ptr)

        pro.close()

        # attention-phase pools
        ptp = ctx.enter_context(tc.tile_pool(name="ptp", bufs=1))
        ppden = ctx.enter_context(tc.tile_pool(name="ppden", bufs=1, space="PSUM"))
        ppo = ctx.enter_context(tc.tile_pool(name="ppo", bufs=2, space="PSUM"))
        ppf = ctx.enter_context(tc.tile_pool(name="ppf", bufs=1, space="PSUM"))

        # ---- attention ----
        pt_big = ptp.tile([P, NJT, IG], f32r)
        for g in range(NIG):
            qs = q_sb[:, g * IG : (g + 1) * IG]

            # S^T chunk-pair matmuls + one exp per 1024 columns, then a
            # pair-sum on DVE/GpSimd so the denominator matmul only needs
            # 16 chunks
            for jp in range(NJT // 2):
                ps = pps.tile([P, 2, IG], f32, tag="ps")
                for h in range(2):
                    jt = 2 * jp + h
                    nc.tensor.matmul(
                        ps[:, h, :],
                        lhsT=k_sb[:, jt * P : (jt + 1) * P],
                        rhs=qs,
                        start=True,
                        stop=True,
                    )
                nc.scalar.activation(
                    out=pt_big[:, 2 * jp : 2 * jp + 2, :],
                    in_=ps,
                    func=AF.Exp,
                    scale=SCALE,
                )

            # denominators: ones-matrix matmul -> every partition holds the sums
            pden = ppden.tile([P, IG], f32)
            for jt in range(NJT):
                nc.tensor.matmul(
                    pden,
                    lhsT=ones_r,
                    rhs=pt_big[:, jt, :],
                    start=(jt == 0),
                    stop=(jt == NJT - 1),
                )

            po = ppo.tile([P, IG], f32)
            for jt in range(NJT):
                nc.tensor.matmul(
                    po,
                    lhsT=vt_sb[:, jt, :],
                    rhs=pt_big[:, jt, :],
                    start=(jt == 0),
                    stop=(jt == NJT - 1),
                )

            bc = bcp.tile([P, IG], f32)
            nc.vector.reciprocal(bc, pden)
            ot = otp.tile([P, IG], f32r)
            nc.vector.tensor_mul(ot, po, bc)

            for oc in range(NCH):
                pf = ppf.tile([P, IG], f32)
                nc.tensor.matmul(pf, lhsT=w_o[:, oc * P : (oc + 1) * P], rhs=ot, start=True, stop=True)
                yf = yfp.tile([P, IG], f32)
                nc.vector.tensor_copy(out=yf, in_=pf)
                nc.sync.dma_start(out=ypv[oc, :, g, :], in_=yf)

    nc.compile()
    return nc


def get_nc():
    if "nc" not in _NC_CACHE:
        _NC_CACHE["nc"] = _build_nc()
    return _NC_CACHE["nc"]


def make_in_maps(inputs):
    x = np.ascontiguousarray(np.asarray(inputs["x"], dtype=np.float32))
    wq = np.asarray(inputs["wq"], np.float32)
    wk = np.asarray(inputs["wk"], np.float32)
    wv = np.asarray(inputs["wv"], np.float32)
    bq = np.asarray(inputs["bq"], np.float32)
    bk = np.asarray(inputs["bk"], np.float32)
    bv = np.asarray(inputs["bv"], np.float32)
    wo = np.asarray(inputs["wo"], np.float32)
    gn_scale = np.asarray(inputs["gn_scale"], np.float32)
    gn_bias = np.asarray(inputs["gn_bias"], np.float32)

    in_maps = []
    for cid in range(8):
        b, h = divmod(cid, HEADS)
        sl = slice(h * HC, (h + 1) * HC)
        in_maps.append(
            {
                "xb": x[b].reshape(C, N),
                "wqt": np.ascontiguousarray(wq[sl, :].T),
                "wkt": np.ascontiguousarray(wk[sl, :].T),
                "wvt": np.ascontiguousarray(wv[sl, :].T),
                "wot": np.ascontiguousarray(wo[:, sl].T),
                "bqh": np.ascontiguousarray(bq[sl].reshape(HC, 1)),
                "bkh": np.ascontiguousarray(bk[sl].reshape(HC, 1)),
                "bvh": np.ascontiguousarray(bv[sl].reshape(HC, 1)),
                "gns": np.ascontiguousarray(gn_scale.reshape(1, C)),
                "gnb": np.ascontiguousarray(gn_bias.reshape(1, C)),
            }
        )
    return in_maps


def assemble_output(inputs, yps):
    x = np.asarray(inputs["x"], np.float32)
    bo = np.asarray(inputs["bo"], np.float32)
    y = x.reshape(2, C, N).astype(np.float32).copy()
    y += bo.reshape(1, C, 1)
    for cid in range(8):
        b = cid // HEADS
        y[b] += yps[cid]
    return y.reshape(2, C, 64, 64)


def run(inputs, trace=False):
    from concourse.bass_utils import run_bass_kernel_spmd

    nc = get_nc()
    in_maps = make_in_maps(inputs)
    res = run_bass_kernel_spmd(nc, in_maps, list(range(8)), trace=trace)
    yps = [r["yp"] for r in res.results]
    return assemble_output(inputs, yps), res


def kernel(**inputs):
    y, _ = run(inputs, trace=False)
    return y
